# revision 22
# baseline (speedup 1.0000x reference)
"""BF15 linear layer for Trainium2, 8-core data-parallel.

Reference semantics:
  y = bf16(bf15(x) @ W.T); y = bf16(fp32(y) + bias)

Strategy (v7):
- Shard x over tokens (32768 -> 8 x 4096), replicate W + bias.
- Hybrid-precision contraction: K = 1024 = 8 subtiles of 128.
  * ko 2..7 (K=768) run in bf16: x ships as bf15-in-bf16 bits (host-masked,
    zero device preprocessing), W rounded to bf16 (RNE).
  * ko 0..1 (K=256) run as TWO fp8 DoubleRow matmuls at 2x PE rate. The
    DR pair slots hold (x8, dx8) where x8 = e4m3(x/4) and dx8 = the exact
    e4m3 residual (bf15 has 7 significant bits = 4 + 3, so the split is
    exact), against W8 = e4m3(4*W) duplicated in both slots. Each cell
    computes x8*W8 + dx8*W8 = bf15(x)*W8 exactly (products fit e10m10),
    so the fp8 slice's only error is W8's e4m3 quantization.
  Measured accuracy vs the fp32 reference: rel_l2 ~ 1.48e-2 (gate 2e-2).
  PE cost per 128x512 output group: 6 bf16 matmuls + 2 DR matmuls
  ~ 1536 ns vs 1728 ns all-bf16.
- Fused epilogue per group: one DVE op y_bf16 = psum_f32 + bias_f32
  reading PSUM directly; store via HWDGE.
- Filler matmuls on a reserved PSUM bank keep the PE/HAM busy during the
  DMA-paced ramp; real groups rotate over 7 banks and are emitted in
  predicted-arrival order (measured landing times + ~1.2us semaphore
  observation latency).
- Queue split: x(+x8) stages ride qSP; W chunks + w8 slices + bias + the
  y stores ride qAct; W chunks 5-7 (+w8 slices) interleave on qSP after
  the early x stages. The last group's stores go to the idle qSP and the
  final piece is only 128 columns wide to shorten the tail chain.
"""

import numpy as np
import ml_dtypes

# Problem shape (hardcoded per contract).
B, S, IN, OUT = 8, 4096, 1024, 4096
N_CORES = 8
M = B * S // N_CORES  # tokens per core = 4096

P = 128
KO8 = 2               # k-subtiles (of 128) done in fp8 DoubleRow
KOB = IN // P - KO8   # 6 bf16 k-subtiles
N_CHUNK = 512
N_CHUNKS = OUT // N_CHUNK  # 8
M_SUB = 128  # tokens per matmul (output partitions)
S8 = 4.0     # fp8 scale: x/S8, W*S8

_NC = {}
LAST_RESULTS = None


def _build():
    from concourse import bacc
    import concourse.mybir as mybir
    import concourse.tile as tile
    from concourse.bass import ds, ts

    f32 = mybir.dt.float32
    bf16 = mybir.dt.bfloat16
    u16 = mybir.dt.uint16
    f8 = mybir.dt.float8e4
    DR = mybir.MatmulPerfMode.DoubleRow

    nc = bacc.Bacc("TRN2", target_bir_lowering=False, debug=False,
                   num_devices=N_CORES)
    xt = nc.dram_tensor("xt", [KOB * P, M], u16, kind="ExternalInput")
    xt8 = nc.dram_tensor("xt8", [KO8 * 2 * P, M], f8, kind="ExternalInput")
    wt = nc.dram_tensor("wt", [KOB * P, OUT], bf16, kind="ExternalInput")
    wt8 = nc.dram_tensor("wt8", [KO8 * 2 * P, OUT], f8, kind="ExternalInput")
    bias = nc.dram_tensor("bias", [OUT], f32, kind="ExternalInput")
    y = nc.dram_tensor("y", [M, OUT], bf16, kind="ExternalOutput")

    xr = xt.ap().rearrange("(ko ki) m -> ki ko m", ki=P)    # [128, 6, M]
    x8r = xt8.ap().rearrange("(p ki) m -> ki p m", ki=P)    # [128, 4, M]
    wr = wt.ap().rearrange("(ko ki) n -> ki ko n", ki=P)    # [128, 6, OUT]
    w8r = wt8.ap().rearrange("(p ki) n -> ki p n", ki=P)    # [128, 4, OUT]
    yr = y.ap()

    # Filler pacing model (us), calibrated from traces.
    FILL_COLD_NS, FILL_WARM_NS = 0.427, 0.216
    GROUP_NS = 2 * 0.120 + KOB * 0.216
    PE_T0 = 7.3
    HAM_WARM_T = 11.2

    # Predicted availability (measured transfer end + ~1.2us sem latency).
    stage_list = [(0, 128), (128, 128), (256, 256)] + \
        [(512 + 512 * i, 512) for i in range((M - 512) // 512)]
    tx_stage = [12.9, 16.0, 20.0, 29.5, 38.5, 48.5, 53.8, 59.1, 64.4, 69.7]
    tw = [16.5, 22.4, 28.3, 34.2, 40.1, 25.4, 33.6, 45.6]
    TW8C0 = 11.9   # w8 chunk-0 slice
    TW0A = 14.2    # W0 bf16 ko 2-4
    sub_stage = []   # sub index -> stage index
    tx_sub = []
    for si, (s0, sz) in enumerate(stage_list):
        for _ in range(sz // M_SUB):
            sub_stage.append(si)
            tx_sub.append(tx_stage[si])
    n_subs = len(tx_sub)
    pairs = [(max(tx_sub[sub], tw[c]), sub, c)
             for sub in range(n_subs) for c in range(N_CHUNKS)]
    pairs.sort(key=lambda t: (t[0], t[1], t[2]))
    order = [(sub, c) for _, sub, c in pairs]

    sub_m0 = []
    for si, (s0, sz) in enumerate(stage_list):
        for j in range(sz // M_SUB):
            sub_m0.append(s0 + j * M_SUB)

    with tile.TileContext(nc) as tc:
        with (
            tc.tile_pool(name="const", bufs=1) as const,
            tc.tile_pool(name="brow", bufs=1) as brow,
            tc.tile_pool(name="yout", bufs=24) as yout,
            tc.tile_pool(name="psum", bufs=1, space="PSUM") as psum,
        ):
            wz = const.tile([P, N_CHUNK], bf16, tag="warm")
            nc.vector.memset(wz[:], 0.0)
            pw = psum.tile([P, N_CHUNK], f32, tag="ps7", name="ps7")

            pe_t = [PE_T0]

            def fill_until(t_avail):
                n = 0
                while pe_t[0] + 0.05 < t_avail and n < 64:
                    nc.tensor.matmul(pw[:], wz[:, :P], wz[:],
                                     start=True, stop=True)
                    pe_t[0] += (FILL_COLD_NS if pe_t[0] < HAM_WARM_T
                                else FILL_WARM_NS)
                    n += 1

            # qAct: bias, w8[c0], W0 in ko-halves, then per-chunk (w8, Wbf16)
            # pairs for chunks 1-4; stores follow.
            bias_row = brow.tile([1, OUT], f32, tag="brow")
            nc.scalar.dma_start(bias_row[:], bias.ap()[None, :])
            bias_sb = const.tile([P, OUT], f32, tag="bias")
            nc.gpsimd.partition_broadcast(bias_sb[:], bias_row[:])

            w_sb = [const.tile([P, KOB, N_CHUNK], bf16, name=f"w{nci}",
                               tag=f"w{nci}") for nci in range(N_CHUNKS)]
            w8_sb = const.tile([P, KO8 * 2, OUT], f8, tag="w8")
            nc.scalar.dma_start(w8_sb[:, :, ts(0, N_CHUNK)],
                                w8r[:, :, ts(0, N_CHUNK)])
            for h in range(2):
                nc.scalar.dma_start(w_sb[0][:, 3 * h:3 * h + 3, :],
                                    wr[:, 3 * h:3 * h + 3, ts(0, N_CHUNK)])
            for nci in (1, 2, 3, 4):
                nc.scalar.dma_start(w8_sb[:, :, ts(nci, N_CHUNK)],
                                    w8r[:, :, ts(nci, N_CHUNK)])
                nc.scalar.dma_start(w_sb[nci][:],
                                    wr[:, :, ts(nci, N_CHUNK)])

            xmm_tiles = [None] * len(stage_list)
            x8mm_tiles = [None] * len(stage_list)

            def load_stage(si):
                s0, sz = stage_list[si]
                xmm = const.tile([P, KOB, sz], u16, name=f"xmm{si}",
                                 tag=f"xmm{si}")
                nc.sync.dma_start(xmm[:], xr[:, :, s0:s0 + sz])
                x8mm = const.tile([P, KO8 * 2, sz], f8, name=f"x8mm{si}",
                                  tag=f"x8mm{si}")
                nc.sync.dma_start(x8mm[:], x8r[:, :, s0:s0 + sz])
                wq = {2: 5, 3: 6, 4: 7}.get(si)
                if wq is not None:  # W chunks 5..7 (+w8) interleave on qSP
                    nc.sync.dma_start(w_sb[wq][:],
                                      wr[:, :, ts(wq, N_CHUNK)])
                    nc.sync.dma_start(w8_sb[:, :, ts(wq, N_CHUNK)],
                                      w8r[:, :, ts(wq, N_CHUNK)])
                xmm_tiles[si] = xmm
                x8mm_tiles[si] = x8mm

            loaded = [False] * len(stage_list)
            t_avail = [max(tx_sub[sub], tw[c]) for _, sub, c in pairs]
            for gi, (sub, nci) in enumerate(order):
                si = sub_stage[sub]
                if not loaded[si]:
                    # keep qSP ahead: issue this and the next stage's load
                    for sj in (si, si + 1):
                        if sj < len(stage_list) and not loaded[sj]:
                            load_stage(sj)
                            loaded[sj] = True
                m0 = sub_m0[sub]
                s0 = stage_list[si][0]
                xmm = xmm_tiles[si]
                x8mm = x8mm_tiles[si]
                ps = psum.tile([P, N_CHUNK], f32, tag=f"ps{gi % 7}",
                               name=f"ps{gi % 7}")
                lhs = xmm[:, :, ds(m0 - s0, M_SUB)].bitcast(bf16)
                lhs8 = x8mm[:, :, ds(m0 - s0, M_SUB)]

                def dr_mm(kp, n0, nh, start):
                    nc.tensor.matmul(
                        ps[:, ds(n0, nh)], lhs8[:, 2 * kp:2 * kp + 2, :],
                        w8_sb[:, 2 * kp:2 * kp + 2,
                              ds(nci * N_CHUNK + n0, nh)],
                        start=start, stop=False, perf_mode=DR)

                def bf_mm(ko, n0, nh, stop):
                    nc.tensor.matmul(
                        ps[:, ds(n0, nh)], lhs[:, ko, :],
                        w_sb[nci][:, ko, ds(n0, nh)],
                        start=False, stop=stop)

                if gi == 0:
                    # pace group 0 along the first W slices, filling gaps
                    fill_until(TW8C0)
                    dr_mm(0, 0, N_CHUNK, True)
                    dr_mm(1, 0, N_CHUNK, False)
                    fill_until(TW0A)
                    for ko in range(3):
                        bf_mm(ko, 0, N_CHUNK, False)
                    fill_until(t_avail[0])
                    for ko in range(3, KOB):
                        bf_mm(ko, 0, N_CHUNK, ko == KOB - 1)
                    pe_t[0] = max(pe_t[0], t_avail[0]) + GROUP_NS / 2
                    ysb = yout.tile([P, N_CHUNK], bf16, tag="ysb512")
                    nc.vector.tensor_tensor(
                        ysb[:], ps[:], bias_sb[:, ts(nci, N_CHUNK)],
                        mybir.AluOpType.add)
                    nc.scalar.dma_start(
                        yr[m0:m0 + M_SUB, ts(nci, N_CHUNK)], ysb[:])
                    continue

                fill_until(t_avail[gi])
                pe_t[0] = max(pe_t[0], t_avail[gi]) + GROUP_NS
                # Split the final group into a 384+128 pair so the very last
                # epilogue chain (DVE add + store) covers only 128 columns.
                pieces = [(0, 384), (384, 128)] if gi == len(order) - 1 \
                    else [(0, N_CHUNK)]
                for n0, nh in pieces:
                    # interleave DR between bf16 MMs so every LDWEIGHTS
                    # hides under the preceding matmul
                    dr_mm(0, n0, nh, True)
                    bf_mm(0, n0, nh, False)
                    dr_mm(1, n0, nh, False)
                    for ko in range(1, KOB):
                        bf_mm(ko, n0, nh, ko == KOB - 1)
                    ysb = yout.tile([P, nh], bf16, tag=f"ysb{nh}",
                                    bufs=2 if nh != N_CHUNK else None)
                    # fused epilogue: bf16(psum_f32 + bias_f32), DVE reads PSUM
                    nc.vector.tensor_tensor(
                        ysb[:], ps[:, ds(n0, nh)],
                        bias_sb[:, ds(nci * N_CHUNK + n0, nh)],
                        mybir.AluOpType.add)
                    # route the tail stores to the idle SP queue so the final
                    # store isn't stuck behind the qAct store backlog
                    eng = nc.sync if gi >= len(order) - 4 else nc.scalar
                    eng.dma_start(
                        yr[m0:m0 + M_SUB, ds(nci * N_CHUNK + n0, nh)],
                        ysb[:])
    nc.compile()
    return nc


def _get_nc():
    if "v7" not in _NC:
        _NC["v7"] = _build()
    return _NC["v7"]


def kernel(x: np.ndarray, weight: np.ndarray, bias: np.ndarray) -> np.ndarray:
    from concourse.bass_utils import run_bass_kernel_spmd

    global LAST_RESULTS
    nc = _get_nc()

    f8 = ml_dtypes.float8_e4m3
    x2d = np.ascontiguousarray(x, dtype=np.float32).reshape(B * S, IN)
    # bf15: keep the top 16 bits of each fp32 and clear the last mantissa
    # bit -> exact bf15 value in a bf16 bit pattern (truncation toward zero).
    xb = ((x2d.view(np.uint32) >> 16) & 0xFFFE).astype(np.uint16)
    # fp8 slice (features 0..255): exact (x8, dx8) split of bf15(x)/S8
    xs = ((xb[:, :KO8 * P].astype(np.uint32) << 16).view(np.float32)) / S8
    x8 = xs.astype(f8)
    dx8 = (xs - x8.astype(np.float32)).astype(f8)

    wt_f = weight.astype(np.float32, copy=False).T  # [IN, OUT]
    wt = np.ascontiguousarray(wt_f[KO8 * P:].astype(ml_dtypes.bfloat16))
    w8 = (wt_f[:KO8 * P] * S8).astype(f8)  # [256, OUT]
    wt8 = np.ascontiguousarray(np.concatenate(
        [w8[0:P], w8[0:P], w8[P:2 * P], w8[P:2 * P]], axis=0))  # [512, OUT]
    bias = np.ascontiguousarray(bias, dtype=np.float32)

    in_maps = []
    for c in range(N_CORES):
        sl = slice(c * M, (c + 1) * M)
        xt8 = np.concatenate([x8[sl, 0:P].T, dx8[sl, 0:P].T,
                              x8[sl, P:2 * P].T, dx8[sl, P:2 * P].T], axis=0)
        in_maps.append({
            "xt": np.ascontiguousarray(xb[sl, KO8 * P:].T),
            "xt8": np.ascontiguousarray(xt8),
            "wt": wt, "wt8": wt8, "bias": bias,
        })

    LAST_RESULTS = run_bass_kernel_spmd(
        nc, in_maps, core_ids=list(range(N_CORES)))
    out = np.concatenate(
        [LAST_RESULTS.results[c]["y"] for c in range(N_CORES)], axis=0)
    return out.reshape(B, S, OUT).astype(ml_dtypes.bfloat16, copy=False)


# revision 23
# speedup vs baseline: 1.0305x; 1.0305x over previous
"""BF15 linear layer for Trainium2, 8-core data-parallel.

Reference semantics:
  y = bf16(bf15(x) @ W.T); y = bf16(fp32(y) + bias)

Strategy:
- Shard x over tokens (32768 -> 8 x 4096), replicate W + bias.
- Host-side prep: x is sliced to its top 16 bits AND bf15-masked (0xFFFE)
  on the host, so the device receives ready-to-multiply bf15-in-bf16 bits
  and needs ZERO preprocessing ops: x stages DMA straight into the resident
  matmul tiles. W is transposed and rounded to bf16 on the host.
- Single bf16 pass: bf15(x) (7 sig bits) is exact in bf16; the only model
  deviation is bf16 rounding of W (~2^-8) and a fused bias add that skips
  the reference's intermediate bf16 rounding. Measured rel_l2 ~ 3.4e-3
  against the fp32 reference (gate is 2e-2).
- Epilogue per group is ONE fused DVE op: y_bf16 = psum_f32 + bias_f32,
  reading PSUM directly (no scalar copy), then the store.
- Filler matmuls on a reserved PSUM bank keep the PE busy (and the HAM
  clock gate open) whenever the schedule predicts a DMA-paced idle
  period; real groups rotate over the other 7 banks and are emitted in
  predicted-arrival order (measured DMA landing times + ~1.2us semaphore
  observation latency), so the PE never starves while inputs stream in.
- Queue split: x stages ride the SP (sync) HWDGE queue; W chunks 0-4 +
  bias ride the ACT (scalar) queue ahead of the y stores; W chunks 5-7
  interleave between x stages on SP. Chunk 0 is split into ko-halves so
  the first matmul group starts as soon as the earliest slice lands
  (PSUM accumulation state survives interleaved fillers, which use a
  different bank).
- Tail: the last group is emitted as a 384+128 column pair and its stores
  go to the idle SP queue, shortening the final DVE+store+semaphore chain
  after the last matmul.
"""

import numpy as np
import ml_dtypes

# Problem shape (hardcoded per contract).
B, S, IN, OUT = 8, 4096, 1024, 4096
N_CORES = 8
M = B * S // N_CORES  # tokens per core = 4096

P = 128
KO = IN // P  # 8 k-subtiles
N_CHUNK = 512
N_CHUNKS = OUT // N_CHUNK  # 8
M_SUB = 128  # tokens per matmul (output partitions)

_NC = {}
LAST_RESULTS = None


def _build():
    from concourse import bacc
    import concourse.mybir as mybir
    import concourse.tile as tile
    from concourse.bass import ds, ts

    f32 = mybir.dt.float32
    bf16 = mybir.dt.bfloat16
    u16 = mybir.dt.uint16

    nc = bacc.Bacc("TRN2", target_bir_lowering=False, debug=False,
                   num_devices=N_CORES)
    xt = nc.dram_tensor("xt", [IN, M], u16, kind="ExternalInput")
    wt = nc.dram_tensor("wt", [IN, OUT], bf16, kind="ExternalInput")
    bias = nc.dram_tensor("bias", [OUT], f32, kind="ExternalInput")
    y = nc.dram_tensor("y", [M, OUT], bf16, kind="ExternalOutput")

    xr = xt.ap().rearrange("(ko ki) m -> ki ko m", ki=P)  # [128, 8, M]
    wr = wt.ap().rearrange("(ko ki) n -> ki ko n", ki=P)  # [128, 8, OUT]
    yr = y.ap()

    # Filler (PE-warm) matmul pacing model: cold rate until the HAM clock
    # gate opens (~3.9us of sustained PE activity), then full rate.
    FILL_COLD_NS, FILL_WARM_NS = 0.427, 0.216  # us per N=512 matmul
    GROUP_NS = 8 * 0.216  # us per 8-matmul group
    PE_T0 = 7.3           # PE free after preamble (us, measured)
    HAM_WARM_T = 11.2     # clock gate opens ~3.9us after fillers start

    # Measured transfer-complete times + ~1.2us DMA-semaphore observation
    # latency (from traces).
    stage_list = [(0, 128), (128, 128), (256, 256)] + \
        [(512 + 512 * i, 512) for i in range((M - 512) // 512)]
    tx_stage = [12.1, 16.3, 20.3, 29.8, 38.9, 48.9, 54.2, 59.5, 64.8, 70.1]
    tw = [16.0, 22.3, 28.4, 38.7, 44.7, 25.2, 32.9, 44.9]
    TW0A = 13.6  # W0's first ko-half (+ sem latency)
    sub_stage = []   # sub index -> stage index
    tx_sub = []
    for si, (s0, sz) in enumerate(stage_list):
        for _ in range(sz // M_SUB):
            sub_stage.append(si)
            tx_sub.append(tx_stage[si])
    n_subs = len(tx_sub)
    pairs = [(max(tx_sub[sub], tw[c]), sub, c)
             for sub in range(n_subs) for c in range(N_CHUNKS)]
    pairs.sort(key=lambda t: (t[0], t[1], t[2]))
    order = [(sub, c) for _, sub, c in pairs]

    sub_m0 = []
    for si, (s0, sz) in enumerate(stage_list):
        for j in range(sz // M_SUB):
            sub_m0.append(s0 + j * M_SUB)

    with tile.TileContext(nc) as tc:
        with (
            tc.tile_pool(name="const", bufs=1) as const,
            tc.tile_pool(name="brow", bufs=1) as brow,
            tc.tile_pool(name="yout", bufs=24) as yout,
            tc.tile_pool(name="psum", bufs=1, space="PSUM") as psum,
        ):
            wz = const.tile([P, N_CHUNK], bf16, tag="warm")
            nc.vector.memset(wz[:], 0.0)
            pw = psum.tile([P, N_CHUNK], f32, tag="ps7", name="ps7")

            pe_t = [PE_T0]

            def fill_until(t_avail):
                # emit fillers bridging predicted PE idle up to t_avail
                n = 0
                while pe_t[0] + 0.05 < t_avail and n < 64:
                    nc.tensor.matmul(pw[:], wz[:, :P], wz[:],
                                     start=True, stop=True)
                    pe_t[0] += (FILL_COLD_NS if pe_t[0] < HAM_WARM_T
                                else FILL_WARM_NS)
                    n += 1

            # bias first on qAct (tiny), then W chunks 0-4.
            bias_row = brow.tile([1, OUT], f32, tag="brow")
            nc.scalar.dma_start(bias_row[:], bias.ap()[None, :])
            bias_sb = const.tile([P, OUT], f32, tag="bias")
            nc.gpsimd.partition_broadcast(bias_sb[:], bias_row[:])

            w_sb = [const.tile([P, KO, N_CHUNK], bf16, name=f"w{nci}",
                               tag=f"w{nci}") for nci in range(N_CHUNKS)]
            # qAct favors few big transfers (~1us fixed cost per DMA): W0 in
            # ko-halves (early slices start the first groups), W1-4 whole.
            for h in range(2):
                nc.scalar.dma_start(w_sb[0][:, 4 * h:4 * h + 4, :],
                                    wr[:, 4 * h:4 * h + 4, ts(0, N_CHUNK)])
            for nci in (1, 2, 3, 4):
                nc.scalar.dma_start(w_sb[nci][:],
                                    wr[:, :, ts(nci, N_CHUNK)])

            xmm_tiles = [None] * len(stage_list)

            def load_stage(si):
                s0, sz = stage_list[si]
                xmm = const.tile([P, KO, sz], u16, name=f"xmm{si}",
                                 tag=f"xmm{si}")
                nc.sync.dma_start(xmm[:], xr[:, :, s0:s0 + sz])
                wq = {2: 5, 3: 6, 4: 7}.get(si)
                if wq is not None:  # W chunks 5..7 interleave on qSP
                    nc.sync.dma_start(w_sb[wq][:],
                                      wr[:, :, ts(wq, N_CHUNK)])
                xmm_tiles[si] = xmm

            loaded = [False] * len(stage_list)
            t_avail = [max(tx_sub[sub], tw[c]) for _, sub, c in pairs]
            for gi, (sub, nci) in enumerate(order):
                si = sub_stage[sub]
                if not loaded[si]:
                    # keep qSP ahead: issue this and the next stage's load
                    for sj in (si, si + 1):
                        if sj < len(stage_list) and not loaded[sj]:
                            load_stage(sj)
                            loaded[sj] = True
                m0 = sub_m0[sub]
                s0 = stage_list[si][0]
                xmm = xmm_tiles[si]
                ps = psum.tile([P, N_CHUNK], f32, tag=f"ps{gi % 7}",
                               name=f"ps{gi % 7}")
                lhs = xmm[:, :, ds(m0 - s0, M_SUB)].bitcast(bf16)
                if gi == 0:
                    # W0 arrives in ko-halves: start the first group on the
                    # early half, bridge the gap with fillers (accumulation
                    # state lives in the bank, so fillers can interleave).
                    fill_until(TW0A)
                    for ko in range(KO // 2):
                        nc.tensor.matmul(
                            ps[:], lhs[:, ko, :], w_sb[0][:, ko, :],
                            start=(ko == 0), stop=False)
                    pe_t[0] = max(pe_t[0], TW0A) + GROUP_NS / 2
                fill_until(t_avail[gi])
                pe_t[0] = max(pe_t[0], t_avail[gi]) + GROUP_NS
                # Split the final group into a 384+128 pair so the very last
                # epilogue chain (DVE add + store) covers only 128 columns.
                pieces = [(0, 384), (384, 128)] if gi == len(order) - 1 \
                    else [(0, N_CHUNK)]
                for n0, nh in pieces:
                    kos = range(KO // 2, KO) if gi == 0 else range(KO)
                    for ko in kos:
                        nc.tensor.matmul(
                            ps[:, ds(n0, nh)], lhs[:, ko, :],
                            w_sb[nci][:, ko, ds(n0, nh)],
                            start=(ko == 0), stop=(ko == KO - 1))
                    ysb = yout.tile([P, nh], bf16, tag=f"ysb{nh}",
                                    bufs=2 if nh != N_CHUNK else None)
                    # fused epilogue: bf16(psum_f32 + bias_f32), DVE reads PSUM
                    nc.vector.tensor_tensor(
                        ysb[:], ps[:, ds(n0, nh)],
                        bias_sb[:, ds(nci * N_CHUNK + n0, nh)],
                        mybir.AluOpType.add)
                    # route the tail stores to the idle SP queue so the final
                    # store isn't stuck behind the qAct store backlog
                    eng = nc.sync if gi >= len(order) - 4 else nc.scalar
                    eng.dma_start(
                        yr[m0:m0 + M_SUB, ds(nci * N_CHUNK + n0, nh)],
                        ysb[:])
    nc.compile()
    return nc


def _get_nc():
    if "v6" not in _NC:
        _NC["v6"] = _build()
    return _NC["v6"]


def kernel(x: np.ndarray, weight: np.ndarray, bias: np.ndarray) -> np.ndarray:
    from concourse.bass_utils import run_bass_kernel_spmd

    global LAST_RESULTS
    nc = _get_nc()

    x2d = np.ascontiguousarray(x, dtype=np.float32).reshape(B * S, IN)
    # bf15: keep the top 16 bits of each fp32 and clear the last mantissa
    # bit -> exact bf15 value in a bf16 bit pattern (truncation toward zero).
    x2d = ((x2d.view(np.uint32) >> 16) & 0xFFFE).astype(np.uint16)
    wt = np.ascontiguousarray(
        weight.astype(np.float32, copy=False).T.astype(ml_dtypes.bfloat16))
    bias = np.ascontiguousarray(bias, dtype=np.float32)

    in_maps = []
    for c in range(N_CORES):
        shard = x2d[c * M:(c + 1) * M]
        in_maps.append({"xt": np.ascontiguousarray(shard.T),
                        "wt": wt, "bias": bias})

    LAST_RESULTS = run_bass_kernel_spmd(
        nc, in_maps, core_ids=list(range(N_CORES)))
    out = np.concatenate(
        [LAST_RESULTS.results[c]["y"] for c in range(N_CORES)], axis=0)
    return out.reshape(B, S, OUT).astype(ml_dtypes.bfloat16, copy=False)
